# revision 1
# baseline (speedup 1.0000x reference)
"""Trainium2 Bass kernel for nn_IlluminationPeakModel (histogram_binning).

Math: the reference computes, per sample b (B=32768, T=1024, K=16):
    inp   = circconv(relu(L), irf)                     # [T], sample-independent
    g     = min(inp / sum(inp), 5)                     # clamp factorizes: pc >= 0
    h     = circconv(g, irf)                           # [T], sample-independent
    shifted[b,t] = pc_b * h[(t - s_b) % T] + c_b       # c_b = (pc_b/sbr_b/T)*sum(irf)
    out[b,k] = pc_b*R[s_b,k] + c_b*colsum_k            # deterministic part (host, f64)
             + sum_t sqrt(shifted[b,t]) * noise[b,t] * cmat[t,k]   # device part

The only heavy work is the noise term: gather rolled-h rows from a small DRAM
table, sqrt(scale*x+bias) on ACT (per-partition scale/bias), multiply by noise
on DVE, PE-transpose, and project onto cmat with PE matmuls. Pure data
parallel over 8 cores (4096 samples each).
"""

import os
import sys

import numpy as np

for _p in ("/opt/trn_rl_repo",):
    if _p not in sys.path and os.path.isdir(_p):
        sys.path.insert(0, _p)

import ml_dtypes  # noqa: E402
import concourse.bass as bass  # noqa: E402
import concourse.tile as tile  # noqa: E402
from concourse import mybir  # noqa: E402
from concourse.masks import make_identity  # noqa: E402
from concourse.vector_clock import ScopedClock  # noqa: E402
from concourse.bass_utils import run_bass_kernel_spmd  # noqa: E402

T = 1024  # time bins
B = 32768  # batch
K = 16  # codes
NCORES = 8
NPC = B // NCORES  # samples per core = 4096
P = 128  # partitions
NTILES = NPC // P  # 32 b-tiles per core
PEAK_FACTOR = 5.0

# dtype configuration:
#   htab: dtype of the rolled-h gather table ("f32" | "bf16")
#   z:    dtype of z = sqrt(shifted)*noise fed to PE ("f32" | "bf16")
#   mm:   matmul mode ("f32" | "f32r" | "bf16"); bf16 requires z == bf16
#   noise: dtype noise is shipped to the device in ("f32" | "bf16")
CFG = {
    "htab": os.environ.get("KCFG_HTAB", "bf16"),
    "z": os.environ.get("KCFG_Z", "f32"),
    "mm": os.environ.get("KCFG_MM", "f32"),
    "noise": os.environ.get("KCFG_NOISE", "f32"),
}

_DT = {"f32": mybir.dt.float32, "bf16": mybir.dt.bfloat16}
_NPDT = {"f32": np.float32, "bf16": ml_dtypes.bfloat16}


class PatchedTC(tile.TileContext):
    """TileContext whose tail drain splits its sem waits into single-wait
    nops; the walrus in this container rejects >1 sync wait on a ctrl
    instruction."""

    def _drain_and_barrier(self, tick_clock, wait_clock):
        nc = self.nc
        collector = nc.sync.nop(nofuse=True, hint="pre_drain_wait_collector")
        wait_clock.add_sem_waits(
            collector.ins, ScopedClock({None: tick_clock.global_clock})
        )
        waits = list(collector.ins.sync_info.on_wait or [])
        if len(waits) > 1:
            collector.ins.sync_info.on_wait = [waits[0]]
            for w in waits[1:]:
                extra = nc.sync.nop(nofuse=True, hint="pre_drain_wait")
                extra.ins.sync_info = mybir.SyncInfo(on_wait=[w], on_update=[])
        nc.sync.drain()
        nc.all_engine_barrier()
        assert self.sems is not None
        popped = nc._tile_sem_poison_stack.pop()
        assert popped is self._sem_poison
        nc.clear_and_free_semaphores(list(self.sems.allocated().values()))
        nc.all_engine_barrier()


def _split_multi_waits(nc):
    """This container's walrus rejects instructions carrying more than one
    sync wait. Hoist all but the last wait of every instruction onto
    single-wait NOPs inserted just before it on the same engine."""
    for f in nc.m.functions:
        for blk in f.blocks:
            il = blk.instructions
            ii = 0
            while ii < len(il):
                inst = il[ii]
                si = getattr(inst, "sync_info", None)
                waits = list(si.on_wait) if si and si.on_wait else []
                if len(waits) > 1:
                    eng = inst.engine
                    for w in waits[:-1]:
                        nop = nc.engines[eng].nop(nofuse=True, hint="wait_split")
                        # nop was appended to the current bb; relocate it
                        for f2 in nc.m.functions:
                            for blk2 in f2.blocks:
                                il2 = blk2.instructions
                                if il2 and il2[-1].name == nop.ins.name and not (
                                    blk2 is blk and len(il2) == ii + 1
                                ):
                                    il2.pop()
                        nop.ins.sync_info = mybir.SyncInfo(on_wait=[w], on_update=[])
                        il.insert(ii, nop.ins)
                        ii += 1
                    si.on_wait = [waits[-1]]
                ii += 1


def _circconv(x, h):
    return np.fft.irfft(np.fft.rfft(x, n=T) * np.fft.rfft(h, n=T), n=T)


def _build_bass():
    """Build the per-core Bass program (identical on all cores)."""
    htab_dt = _DT[CFG["htab"]]
    z_dt = _DT[CFG["z"]]
    noise_dt = _DT[CFG["noise"]]
    mm = CFG["mm"]
    mm_dt = mybir.dt.bfloat16 if mm == "bf16" else mybir.dt.float32
    grp = 2 if mm == "f32r" else 1  # f32r needs moving free dim >= 256

    nc = bass.Bass("TRN2", target_bir_lowering=False, debug=False)

    noise_d = nc.dram_tensor("noise", [NPC, T], noise_dt, kind="ExternalInput")
    htab_d = nc.dram_tensor("htab", [T, T], htab_dt, kind="ExternalInput")
    cmat_d = nc.dram_tensor("cmat", [T, K], mm_dt, kind="ExternalInput")
    sidx_d = nc.dram_tensor("sidx", [P, NTILES], mybir.dt.int32, kind="ExternalInput")
    pc_d = nc.dram_tensor("pc", [P, NTILES], mybir.dt.float32, kind="ExternalInput")
    cc_d = nc.dram_tensor("cc", [P, NTILES], mybir.dt.float32, kind="ExternalInput")
    dett_d = nc.dram_tensor("dett", [K, NPC], mybir.dt.float32, kind="ExternalInput")
    outt_d = nc.dram_tensor("outt", [K, NPC], mybir.dt.float32, kind="ExternalOutput")

    with PatchedTC(nc) as tc:
        with (
            tc.tile_pool(name="const", bufs=1) as const,
            tc.tile_pool(name="work", bufs=4) as work,
            tc.tile_pool(name="ztsb", bufs=3) as ztsb_pool,
            tc.tile_pool(name="zt_ps", bufs=4, space="PSUM") as zt_ps_pool,
            tc.tile_pool(name="out_ps", bufs=4, space="PSUM") as out_ps_pool,
        ):
            # ---- constants ----
            ident = const.tile([P, P], z_dt)
            make_identity(nc, ident[:])
            # cmat [T, K] -> sbuf [128, 8, K]; chunk c = rows c*128..c*128+127
            cmat_sb = const.tile([P, T // P, K], mm_dt)
            nc.gpsimd.dma_start(
                out=cmat_sb[:], in_=cmat_d.rearrange("(c p) k -> p c k", p=P)
            )
            sidx_sb = const.tile([P, NTILES], mybir.dt.int32)
            nc.sync.dma_start(out=sidx_sb[:], in_=sidx_d[:, :])
            pc_sb = const.tile([P, NTILES], mybir.dt.float32)
            nc.sync.dma_start(out=pc_sb[:], in_=pc_d[:, :])
            cc_sb = const.tile([P, NTILES], mybir.dt.float32)
            nc.sync.dma_start(out=cc_sb[:], in_=cc_d[:, :])
            dett_sb = const.tile([K, NPC], mybir.dt.float32)
            nc.sync.dma_start(out=dett_sb[:], in_=dett_d[:, :])

            for g0 in range(0, NTILES, grp):
                zt_sb = ztsb_pool.tile([P, T // P, grp * P], mm_dt, tag="zt")
                for g in range(grp):
                    i = g0 + g
                    noise_t = work.tile([P, T], noise_dt, tag="noise")
                    nc.sync.dma_start(out=noise_t[:], in_=noise_d[i * P : (i + 1) * P, :])
                    hroll = work.tile([P, T], htab_dt, tag="hroll")
                    nc.gpsimd.indirect_dma_start(
                        out=hroll[:],
                        out_offset=None,
                        in_=htab_d[:, :],
                        in_offset=bass.IndirectOffsetOnAxis(
                            ap=sidx_sb[:, i : i + 1], axis=0
                        ),
                    )
                    # sqs = sqrt(pc * hroll + c), per-partition scale/bias
                    sqs = work.tile([P, T], noise_dt, tag="sqs")
                    nc.scalar.activation(
                        sqs[:],
                        hroll[:],
                        mybir.ActivationFunctionType.Sqrt,
                        bias=cc_sb[:, i : i + 1],
                        scale=pc_sb[:, i : i + 1],
                    )
                    z = work.tile([P, T], z_dt, tag="z")
                    nc.vector.tensor_mul(z[:], sqs[:], noise_t[:])
                    # transpose 8 chunks of z via PE, 4 at a time into one
                    # PSUM bank, then copy to SBUF (alternate ACT/DVE).
                    for c2 in range(2):
                        zt_ps = zt_ps_pool.tile([P, 4 * P], mybir.dt.float32, tag="ztp")
                        for c4 in range(4):
                            c = c2 * 4 + c4
                            nc.tensor.transpose(
                                out=zt_ps[:, c4 * P : (c4 + 1) * P],
                                in_=z[:, c * P : (c + 1) * P],
                                identity=ident[:],
                            )
                        dst = zt_sb[:, c2 * 4 : (c2 + 1) * 4, g * P : (g + 1) * P]
                        if c2 == 0:
                            nc.scalar.copy(dst, zt_ps[:].rearrange("p (c t) -> p c t", c=4))
                        else:
                            nc.vector.tensor_copy(dst, zt_ps[:].rearrange("p (c t) -> p c t", c=4))
                # project: outT[k, b] += cmat_chunk.T @ ztT_chunk over 8 chunks
                out_ps = out_ps_pool.tile([K, grp * P], mybir.dt.float32, tag="op")
                for c in range(T // P):
                    lhsT = cmat_sb[:, c, :]
                    rhs = zt_sb[:, c, :]
                    if mm == "f32r":
                        lhsT = lhsT.bitcast(mybir.dt.float32r)
                        rhs = rhs.bitcast(mybir.dt.float32r)
                    nc.tensor.matmul(
                        out=out_ps[:],
                        lhsT=lhsT,
                        rhs=rhs,
                        start=(c == 0),
                        stop=(c == T // P - 1),
                    )
                out_sb = work.tile([K, grp * P], mybir.dt.float32, tag="osb")
                nc.vector.tensor_add(
                    out_sb[:], out_ps[:], dett_sb[:, g0 * P : (g0 + grp) * P]
                )
                nc.sync.dma_start(out=outt_d[:, g0 * P : (g0 + grp) * P], in_=out_sb[:])

    _split_multi_waits(nc)
    return nc


def _prepare(learnable_input, irf, cmat, noise_unit, photon_counts, sbrs, bins):
    """Host-side prep: small f64 precompute + per-core input maps."""
    L = np.maximum(np.asarray(learnable_input, dtype=np.float64).reshape(T), 0.0)
    irf64 = np.asarray(irf, dtype=np.float64).reshape(T)
    cmat64 = np.asarray(cmat, dtype=np.float64)
    inp = _circconv(L, irf64)
    area = inp.sum()
    g = np.minimum(inp / area, PEAK_FACTOR)
    h = _circconv(g, irf64)
    sumirf = irf64.sum()

    t_idx = np.arange(T)
    htab = h[(t_idx[None, :] - t_idx[:, None]) % T]  # htab[s, t] = h[(t-s)%T]
    R = htab @ cmat64  # [T, K]
    colsum = cmat64.sum(axis=0)  # [K]

    shifts = (np.asarray(bins).astype(np.int64) % T).astype(np.int32)  # [B]
    pc = np.asarray(photon_counts, dtype=np.float64)
    amb = pc / np.asarray(sbrs, dtype=np.float64) / T
    cadd = amb * sumirf
    det = pc[:, None] * R[shifts] + cadd[:, None] * colsum[None, :]  # [B, K] f64

    htab_dev = htab.astype(_NPDT[CFG["htab"]])
    mm_np = ml_dtypes.bfloat16 if CFG["mm"] == "bf16" else np.float32
    cmat_dev = cmat64.astype(mm_np)
    noise_np = np.asarray(noise_unit)
    if CFG["noise"] == "bf16":
        noise_np = noise_np.astype(ml_dtypes.bfloat16)
    else:
        noise_np = noise_np.astype(np.float32)

    pc32 = pc.astype(np.float32)
    cadd32 = cadd.astype(np.float32)

    in_maps = []
    for core in range(NCORES):
        s = slice(core * NPC, (core + 1) * NPC)
        in_maps.append(
            {
                "noise": np.ascontiguousarray(noise_np[s]),
                "htab": htab_dev,
                "cmat": cmat_dev,
                "sidx": np.ascontiguousarray(shifts[s].reshape(NTILES, P).T),
                "pc": np.ascontiguousarray(pc32[s].reshape(NTILES, P).T),
                "cc": np.ascontiguousarray(cadd32[s].reshape(NTILES, P).T),
                "dett": np.ascontiguousarray(det[s].astype(np.float32).T),
            }
        )
    return in_maps


def run_with_stats(trace=False, **inputs):
    in_maps = _prepare(**inputs)
    nc = _build_bass()
    try:
        res = run_bass_kernel_spmd(
            nc, in_maps, core_ids=list(range(NCORES)), trace=trace
        )
    except ModuleNotFoundError:
        # no axon NTFF hook in this container — run untraced
        res = run_bass_kernel_spmd(
            nc, in_maps, core_ids=list(range(NCORES)), trace=False
        )
    out = np.empty((B, K), dtype=np.float32)
    for core in range(NCORES):
        out[core * NPC : (core + 1) * NPC] = res.results[core]["outt"].T
    return out, res


def kernel(**inputs):
    trace = os.environ.get("KERNEL_TRACE", "0") == "1"
    out, _res = run_with_stats(trace=trace, **inputs)
    return out

